# revision 14
# baseline (speedup 1.0000x reference)
"""Trainium2 Bass kernel for nn_Actor_network (moe_routing).

Data-parallel over 8 NeuronCores: each core processes 256 of the 2048 images
through convs (+pools) -> FC -> 2-expert routing -> softmax.

v3 design:
  - conv1 and conv2 are LINEAR back-to-back (no activation between), so they
    are composed on host into one 5x5 conv (6->18 ch), kernel K12.
  - The composed conv runs as banded-Toeplitz matmuls with kx-PAIR
    replication: K = 2 kx-taps x 10 input rows x 6 ch = 120; the remaining
    kx taps reuse the same SBUF rows at shifted free offsets, giving 3
    accumulating passes (taps {0,1}, {2,3}, {4}).  R=6 output rows per
    window, M = 118 (pair-parity grouped: even-of-pair rows at partition 0,
    odd-of-pair at 64).
  - Each pass is split into even-x / odd-x strided matmuls accumulating in
    two PSUM tiles; horizontal maxpool = ACT copy (psum->SBUF, HW allows
    only one PSUM operand per tensor_tensor) + one DVE tensor_max;
    vertical maxpool = ONE DVE tensor_max with partition-offset operands
    (rows pair-align inside each window since R=6 is even) -> mv tiles.
  - conv3 windows are assembled from mv tiles by span DMA gathers.
  - conv4: 6 passes (3 kx x 2 x-parity) accumulate into ONE psum tile =
    horizontal avgpool for free (0.25 folded into conv4 weights); vertical
    avgpool is folded into the l1 contraction (weights duplicated per row
    pair); pooled features go straight from conv4 psum into persistent
    per-window FC tiles.
  - bf16 activations/weights everywhere in the conv pipeline (PSUM f32).
"""
import numpy as np
import ml_dtypes

import concourse.bass as bass
import concourse.mybir as mybir
from concourse import bacc, tile
from concourse.bass_utils import run_bass_kernel_spmd

F32 = mybir.dt.float32
F32R = mybir.dt.float32r
BF16 = mybir.dt.bfloat16
NP_BF16 = ml_dtypes.bfloat16

N_CORES = 8
B_CORE = 256          # images per core
BLK = 32              # images per pipeline block
HALF = 16             # images per conv12/conv3 matmul
NBLK = B_CORE // BLK  # 8 blocks

NW12 = 10             # conv12 windows: in rows 6w..6w+9 -> out rows 6w..6w+5
C3_WINS = [(w, 5 * w, 7, 5) for w in range(5)] + [(5, 25, 5, 3)]
C4_WINS = [(w, 5 * w, 7, 5) for w in range(5)] + [(5, 25, 3, 1)]


# ---------------------------------------------------------------- host prep
def _compose12(c1_w, c2_w):
    """5x5 composite kernel K12[o,i,s,t] of conv2(conv1(x)) (both valid 3x3)."""
    K12 = np.zeros((18, 6, 5, 5), np.float64)
    for a in range(3):
        for ax in range(3):
            for b in range(3):
                for bx in range(3):
                    K12[:, :, a + b, ax + bx] += np.einsum(
                        "oc,ci->oi", c2_w[:, :, a, ax].astype(np.float64),
                        c1_w[:, :, b, bx].astype(np.float64))
    return K12.astype(np.float32)


def _parity_col(r, co):
    """M-column for output row r (0..5), channel co: pair-parity layout."""
    return (r // 2) * 18 + co + (64 if r % 2 else 0)


def _t12(K12, taps):
    """lhsT [len(taps)*60, 118] for the composed conv: row g*60+u*6+ci holds
    tap taps[g] (polyphase: g also selects the shifted input copy); band
    0 <= u-r <= 4; cols pair-parity grouped."""
    T = np.zeros((len(taps) * 60, 118), np.float32)
    for g, t in enumerate(taps):
        for u in range(10):
            for r in range(6):
                s = u - r
                if 0 <= s < 5:
                    T[g * 60 + u * 6:g * 60 + (u + 1) * 6,
                      _parity_col(r, 0):_parity_col(r, 0) + 18] = K12[:, :, s, t].T
    return T


def _toeplitz(w, kx, U, R, cin):
    T = np.zeros((U * cin, R * 18), np.float32)
    for u in range(U):
        for r in range(R):
            ky = u - r
            if 0 <= ky < 3:
                T[u * cin:(u + 1) * cin, r * 18:(r + 1) * 18] = w[:, :, ky, kx].T
    return T


def _pack3(w, U, R, cin):
    return np.concatenate([_toeplitz(w, kx, U, R, cin) for kx in range(3)], axis=1)


def _host_prep(c1_w, c1_b, c2_w, c2_b, c3_w, c3_b, c4_w, c4_b,
               l1_w, l1_b, l2_w, l2_b, ag_w, ag_b):
    p = {}
    K12 = _compose12(c1_w, c2_w)
    # polyphase tile row blocks: [0:60] = plane shifted +1, [60:120] = plain
    p["Te"] = _t12(K12, [2, 0])    # [120, 118]  same-parity plane taps
    p["To"] = _t12(K12, [3, 1])    # [120, 118]  opposite-plane taps
    p["Tc"] = _t12(K12, [4])       # [60, 118]   tap 4 (shifted block @ +1)

    p["T3"] = _pack3(c3_w, 7, 5, 18)          # [126, 270]
    p["T3p"] = _pack3(c3_w, 5, 3, 18)         # [90, 162]
    p["T4"] = _pack3(c4_w, 7, 5, 18) * 0.25   # [126, 270]
    p["T4p"] = _pack3(c4_w, 3, 1, 18) * 0.25  # [54, 54]

    # l1 weights with vertical avgpool folded in
    wl1 = l1_w[:, :3042].reshape(64, 18, 13, 13)
    for w4, _, _, R4 in C4_WINS:
        Wf = np.zeros((R4 * 18, 13, 64), np.float32)
        for r in range(R4):
            y = 5 * w4 + r
            for x2 in range(13):
                Wf[r * 18:(r + 1) * 18, x2, :] = wl1[:, :, y // 2, x2].T
        p[f"Wfc{w4}"] = Wf.reshape(R4 * 18, 13 * 64)
    p["Wst"] = np.ascontiguousarray(l1_w[:, 3042:3044].astype(np.float32).T)

    # fold conv biases into l1 bias (linear chain; constants commute w/ pools)
    c1 = c1_b.astype(np.float64)
    c2 = c2_b + c2_w.sum((2, 3)).astype(np.float64) @ c1
    c3 = c3_b + c3_w.sum((2, 3)).astype(np.float64) @ c2
    c4 = c4_b + c4_w.sum((2, 3)).astype(np.float64) @ c3
    beff = l1_b.astype(np.float64) + l1_w[:, :3042].reshape(64, 18, 169).sum(-1).astype(np.float64) @ c4
    p["beff"] = beff.astype(np.float32).reshape(64, 1)
    p["Wl2"] = np.ascontiguousarray(l2_w.T.astype(np.float32))
    p["bl2"] = l2_b.astype(np.float32).reshape(64, 1)
    p["We"] = np.ascontiguousarray(ag_w.transpose(2, 0, 1).reshape(64, 10)).astype(np.float32)
    p["be0"] = ag_b[0].astype(np.float32).reshape(5, 1)
    p["be1"] = ag_b[1].astype(np.float32).reshape(5, 1)
    p["ones5"] = np.ones((5, 1), np.float32)
    p["rep5"] = np.ones((1, 5), np.float32)
    for k in ["Te", "To", "Tc", "T3", "T3p", "T4", "T4p",
              "Wfc0", "Wfc1", "Wfc2", "Wfc3", "Wfc4", "Wfc5", "Wst"]:
        p[k] = p[k].astype(NP_BF16)
    return p


# ---------------------------------------------------------------- device build
_CACHE = {}

_WSPEC = [
    ("Te", [120, 118], BF16), ("To", [120, 118], BF16),
    ("Tc", [60, 118], BF16),
    ("T3", [126, 270], BF16), ("T3p", [90, 162], BF16),
    ("T4", [126, 270], BF16), ("T4p", [54, 54], BF16),
    ("Wfc0", [90, 832], BF16), ("Wfc1", [90, 832], BF16),
    ("Wfc2", [90, 832], BF16), ("Wfc3", [90, 832], BF16),
    ("Wfc4", [90, 832], BF16), ("Wfc5", [18, 832], BF16),
    ("Wst", [2, 64], BF16),
    ("Wl2", [64, 64], F32R), ("We", [64, 10], F32R),
    ("beff", [64, 1], F32), ("bl2", [64, 1], F32),
    ("be0", [5, 1], F32), ("be1", [5, 1], F32),
    ("ones5", [5, 1], F32R), ("rep5", [1, 5], F32R),
    ("st", [2, B_CORE], BF16), ("sel", [1, B_CORE], F32R),
]


def _build():
    nc = bacc.Bacc("TRN2", debug=False)

    x_ext = nc.declare_dram_parameter("x", [NW12 * 240, B_CORE, 32], BF16,
                                      isOutput=False)
    wparams = {}
    for name, shape, dt in _WSPEC:
        wparams[name] = nc.declare_dram_parameter(name, shape, dt, isOutput=False)
    out_ext = nc.declare_dram_parameter("out", [B_CORE * 5], F32, isOutput=True)

    with tile.TileContext(nc) as tc:
        with (
            tc.tile_pool(name="wp", bufs=1) as wp,
            tc.tile_pool(name="act", bufs=1) as ap_,
            tc.tile_pool(name="ps", bufs=1, space="PSUM") as ps,
        ):
            W = {}
            for name, ext in wparams.items():
                t = wp.tile(list(ext.shape), ext.dtype, name=f"w_{name}")
                nc.sync.dma_start(out=t[...], in_=ext.ap())
                W[name] = t

            fc = {}
            for w4, _, _, R4 in C4_WINS:
                fc[w4] = wp.tile([R4 * 18, 13, B_CORE], BF16, name=f"fc{w4}")

            for blk in range(NBLK):
                ib = slice(blk * BLK, (blk + 1) * BLK)
                # ---- conv12 window loads (polyphase even/odd planes)
                xw = {}
                for w in range(NW12):
                    te = ap_.tile([120, BLK, 32], BF16, name="xwe", tag="xwe", bufs=4)
                    to = ap_.tile([120, BLK, 32], BF16, name="xwo", tag="xwo", bufs=4)
                    nc.sync.dma_start(out=te[...],
                                      in_=x_ext.ap()[w * 240:w * 240 + 120, ib, :])
                    nc.sync.dma_start(out=to[...],
                                      in_=x_ext.ap()[w * 240 + 120:w * 240 + 240, ib, :])
                    xw[w] = (te, to)

                # ---- conv12 matmuls + maxH + maxV
                mv = {}
                for w in range(NW12):
                    mh = ap_.tile([118, BLK, 30], BF16, name="mh", tag="mh", bufs=3)
                    for s in range(2):
                        isl = slice(s * HALF, (s + 1) * HALF)
                        pse = ps.tile([118, HALF, 30], F32, name="ps2e",
                                      tag="psE", bufs=2)
                        pso = ps.tile([118, HALF, 30], F32, name="ps2o",
                                      tag="psO", bufs=2)
                        xe, xo = xw[w]
                        for pt, same, opp, off in ((pse, xe, xo, 0),
                                                   (pso, xo, xe, 1)):
                            nc.tensor.matmul(
                                pt[...], W["Te"][...],
                                same[:, isl, 0:30], start=True, stop=False)
                            nc.tensor.matmul(
                                pt[...], W["To"][...],
                                opp[:, isl, off:off + 30],
                                start=False, stop=False)
                            nc.tensor.matmul(
                                pt[...], W["Tc"][...],
                                same[0:60, isl, 1:31],
                                start=False, stop=True)
                        se = ap_.tile([118, HALF, 30], F32, name="se",
                                      tag="se", bufs=3)
                        nc.scalar.copy(se[...], pse[...])
                        nc.vector.tensor_max(mh[:, isl, :], se[...], pso[...])
                    mho = ap_.tile([54, BLK, 30], BF16, name="mho",
                                   tag="mho", bufs=3)
                    nc.vector.tensor_copy(mho[...], mh[64:118, :, :])
                    mvw = ap_.tile([54, BLK, 30], BF16, name="mv",
                                   tag="mv", bufs=11)
                    nc.vector.tensor_max(mvw[...], mh[0:54, :, :], mho[...])
                    mv[w] = mvw

                # ---- assemble conv3 windows from mv (span gathers), conv3
                c4win = {}
                for w3, prow0, U, R in C3_WINS:
                    c4win[w3] = ap_.tile([126, BLK, 2, 14], BF16, name="c4w",
                                         tag="c4win", bufs=7)
                ge = 0
                for w3, prow0, U, R in C3_WINS:
                    t3 = ap_.tile([126, BLK, 30], BF16, name="c3w",
                                  tag="c3win", bufs=3)
                    u = 0
                    while u < U:
                        k = prow0 + u
                        mw, r = k // 3, k % 3
                        span = 1
                        while u + span < U and (k + span) // 3 == mw:
                            span += 1
                        eng = nc.sync if ge % 2 == 0 else nc.scalar
                        ge += 1
                        eng.dma_start(
                            out=t3[u * 18:(u + span) * 18, :, :],
                            in_=mv[mw][r * 18:(r + span) * 18, :, :])
                        u += span
                    TW = W["T3"] if U == 7 else W["T3p"]
                    for h in range(2):
                        hsl = slice(h * HALF, (h + 1) * HALF)
                        pt = ps.tile([R * 18, HALF, 28], F32, name="ps3",
                                     tag="psA", bufs=2)
                        for kx in range(3):
                            nc.tensor.matmul(
                                pt[...],
                                TW[0:U * 18, kx * R * 18:(kx + 1) * R * 18],
                                t3[0:U * 18, hsl, kx:kx + 28],
                                start=(kx == 0), stop=(kx == 2))
                        nc.scalar.copy(c4win[w3][0:R * 18, hsl, :, :],
                                       pt[...].rearrange("p i (x two) -> p i two x", two=2))
                for w4 in range(5):
                    src = c4win[w4 + 1] if w4 < 4 else c4win[5]
                    nc.gpsimd.dma_start(out=c4win[w4][90:126, :, :, :],
                                        in_=src[0:36, :, :, :])

                # ---- conv4: 6 passes -> psum IS avgH -> copy into FC tiles
                for w4, _, U, R in C4_WINS:
                    TW = W["T4"] if U == 7 else W["T4p"]
                    pt = ps.tile([R * 18, BLK, 13], F32, name="ps4",
                                 tag="psE", bufs=2)
                    n = 0
                    for j in (0, 1):
                        for kx in range(3):
                            plane = (j + kx) % 2
                            off = (j + kx) // 2
                            rhs = c4win[w4][0:U * 18, :, plane, off:off + 13]
                            nc.tensor.matmul(
                                pt[...],
                                TW[0:U * 18, kx * R * 18:(kx + 1) * R * 18],
                                rhs, start=(n == 0), stop=(n == 5))
                            n += 1
                    nc.vector.tensor_copy(fc[w4][:, :, ib],
                                          pt[...].rearrange("p i x -> p x i"))

            # ---------------- FC + routing (all 256 images)
            ph1 = ps.tile([64, B_CORE], F32, name="ph1", tag="psA", bufs=2)
            first = True
            for w4, _, _, R4 in C4_WINS:
                wfc = W[f"Wfc{w4}"][...].rearrange("p (x m) -> p x m", m=64)
                for x2 in range(13):
                    nc.tensor.matmul(ph1[...], wfc[0:R4 * 18, x2, :],
                                     fc[w4][:, x2, :], start=first, stop=False)
                    first = False
            nc.tensor.matmul(ph1[...], W["Wst"][...], W["st"][...],
                             start=False, stop=True)
            h1 = wp.tile([64, B_CORE], F32R, name="h1")
            nc.scalar.activation(h1[...], ph1[...],
                                 mybir.ActivationFunctionType.Tanh,
                                 bias=W["beff"][...])

            ph2 = ps.tile([64, B_CORE], F32, name="ph2", tag="psA", bufs=2)
            nc.tensor.matmul(ph2[...], W["Wl2"][...], h1[...], start=True, stop=True)
            h2 = wp.tile([64, B_CORE], F32R, name="h2")
            nc.scalar.activation(h2[...], ph2[...],
                                 mybir.ActivationFunctionType.Tanh,
                                 bias=W["bl2"][...])

            We_r = W["We"][...].rearrange("p (e m) -> p e m", m=5)
            g = []
            for e in range(2):
                pe = ps.tile([5, B_CORE], F32, name=f"pe{e}", tag="psO", bufs=2)
                nc.tensor.matmul(pe[...], We_r[:, e, :], h2[...],
                                 start=True, stop=True)
                gt = wp.tile([5, B_CORE], F32, name=f"g{e}")
                nc.scalar.activation(gt[...], pe[...],
                                     mybir.ActivationFunctionType.Identity,
                                     bias=W[f"be{e}"][...])
                g.append(gt)

            psel = ps.tile([5, B_CORE], F32, name="psel", tag="psO", bufs=2)
            nc.tensor.matmul(psel[...], W["rep5"][...], W["sel"][...],
                             start=True, stop=True)
            sel5 = wp.tile([5, B_CORE], F32, name="sel5")
            nc.scalar.copy(sel5[...], psel[...])

            dif = wp.tile([5, B_CORE], F32, name="dif")
            nc.vector.tensor_sub(dif[...], g[1][...], g[0][...])
            nc.vector.tensor_mul(dif[...], dif[...], sel5[...])
            lg = wp.tile([5, B_CORE], F32, name="lg")
            nc.vector.tensor_add(lg[...], g[0][...], dif[...])

            E = wp.tile([5, B_CORE], F32R, name="E")
            nc.scalar.activation(E[...], lg[...], mybir.ActivationFunctionType.Exp)
            psum_s = ps.tile([1, B_CORE], F32, name="psum_s", tag="psO", bufs=2)
            nc.tensor.matmul(psum_s[...], W["ones5"][...], E[...],
                             start=True, stop=True)
            s_sb = wp.tile([1, B_CORE], F32, name="s_sb")
            nc.scalar.copy(s_sb[...], psum_s[...])
            r_sb = wp.tile([1, B_CORE], F32R, name="r_sb")
            with nc.allow_low_precision(reason="f32r reciprocal feeding f32r matmul"):
                nc.vector.reciprocal(r_sb[...], s_sb[...])
            pr5 = ps.tile([5, B_CORE], F32, name="pr5", tag="psO", bufs=2)
            nc.tensor.matmul(pr5[...], W["rep5"][...], r_sb[...],
                             start=True, stop=True)
            r5 = wp.tile([5, B_CORE], F32, name="r5")
            nc.scalar.copy(r5[...], pr5[...])
            probs = wp.tile([5, B_CORE], F32, name="probs")
            nc.vector.tensor_mul(probs[...], E[...], r5[...])

            nc.sync.dma_start(
                out=out_ext.ap().rearrange("(b o) -> o b", o=5), in_=probs[...])

    nc.finalize()
    return nc


def _get_nc():
    if "nc" not in _CACHE:
        _CACHE["nc"] = _build()
    return _CACHE["nc"]


# ---------------------------------------------------------------- input prep
def make_x2(states_core):
    """[2400, B_CORE, 32] bf16 polyphase: window w block = [xeP | xoP], each
    [120]: rows 0:60 = plane, rows 60:120 = plane shifted by one element."""
    B = states_core.shape[0]
    out = np.zeros((NW12 * 240, B, 32), dtype=NP_BF16)
    for w in range(NW12):
        sub = states_core[:, :, 6 * w:6 * w + 10, :]           # [B,6,10,64]
        sub = sub.transpose(2, 1, 0, 3)                        # [10,6,B,64]
        for pi, plane in ((0, sub[..., 0::2]), (1, sub[..., 1::2])):
            base = w * 240 + pi * 120
            pl = plane.reshape(60, B, 32).astype(NP_BF16)
            out[base:base + 60, :, 0:31] = pl[:, :, 1:32]   # shifted block
            out[base + 60:base + 120] = pl                   # plain block
    return out


def make_in_maps(p, states, scores, times, agents_np):
    in_maps = []
    for c in range(N_CORES):
        sl = slice(c * B_CORE, (c + 1) * B_CORE)
        m = dict(p)
        m["x"] = make_x2(states[sl])
        m["st"] = np.stack([scores[sl, 0], times[sl, 0]], axis=0).astype(NP_BF16)
        m["sel"] = agents_np[sl].astype(np.float32).reshape(1, B_CORE)
        in_maps.append(m)
    return in_maps


# ---------------------------------------------------------------- entry point
def kernel(states, scores, times, agents,
           c1_w, c1_b, c2_w, c2_b, c3_w, c3_b, c4_w, c4_b,
           l1_w, l1_b, l2_w, l2_b, ag_w, ag_b, _want_trace=False, **_ignore):
    states = np.asarray(states, np.float32)
    scores = np.asarray(scores, np.float32)
    times = np.asarray(times, np.float32)
    agents_np = np.asarray(agents)
    p = _host_prep(np.asarray(c1_w, np.float32), np.asarray(c1_b, np.float32),
                   np.asarray(c2_w, np.float32), np.asarray(c2_b, np.float32),
                   np.asarray(c3_w, np.float32), np.asarray(c3_b, np.float32),
                   np.asarray(c4_w, np.float32), np.asarray(c4_b, np.float32),
                   np.asarray(l1_w, np.float32), np.asarray(l1_b, np.float32),
                   np.asarray(l2_w, np.float32), np.asarray(l2_b, np.float32),
                   np.asarray(ag_w, np.float32), np.asarray(ag_b, np.float32))

    nc = _get_nc()
    in_maps = make_in_maps(p, states, scores, times, agents_np)

    res = run_bass_kernel_spmd(nc, in_maps, list(range(N_CORES)),
                               trace=_want_trace)
    out = np.concatenate([res.results[c]["out"] for c in range(N_CORES)])
    if _want_trace:
        kernel._last = res
    return out


# revision 16
# speedup vs baseline: 1.0361x; 1.0361x over previous
"""Trainium2 Bass kernel for nn_Actor_network (moe_routing).

Data-parallel over 8 NeuronCores: each core processes 256 of the 2048 images
through convs (+pools) -> FC -> 2-expert routing -> softmax.

v3 design:
  - conv1 and conv2 are LINEAR back-to-back (no activation between), so they
    are composed on host into one 5x5 conv (6->18 ch), kernel K12.
  - The composed conv runs as banded-Toeplitz matmuls with kx-PAIR
    replication: K = 2 kx-taps x 10 input rows x 6 ch = 120; the remaining
    kx taps reuse the same SBUF rows at shifted free offsets, giving 3
    accumulating passes (taps {0,1}, {2,3}, {4}).  R=6 output rows per
    window, M = 118 (pair-parity grouped: even-of-pair rows at partition 0,
    odd-of-pair at 64).
  - Each pass is split into even-x / odd-x strided matmuls accumulating in
    two PSUM tiles; horizontal maxpool = ACT copy (psum->SBUF, HW allows
    only one PSUM operand per tensor_tensor) + one DVE tensor_max;
    vertical maxpool = ONE DVE tensor_max with partition-offset operands
    (rows pair-align inside each window since R=6 is even) -> mv tiles.
  - conv3 windows are assembled from mv tiles by span DMA gathers.
  - conv4: 6 passes (3 kx x 2 x-parity) accumulate into ONE psum tile =
    horizontal avgpool for free (0.25 folded into conv4 weights); vertical
    avgpool is folded into the l1 contraction (weights duplicated per row
    pair); pooled features go straight from conv4 psum into persistent
    per-window FC tiles.
  - bf16 activations/weights everywhere in the conv pipeline (PSUM f32).
"""
import numpy as np
import ml_dtypes

import concourse.bass as bass
import concourse.mybir as mybir
from concourse import bacc, tile
from concourse.bass_utils import run_bass_kernel_spmd

F32 = mybir.dt.float32
F32R = mybir.dt.float32r
BF16 = mybir.dt.bfloat16
NP_BF16 = ml_dtypes.bfloat16

N_CORES = 8
B_CORE = 256          # images per core
BLK = 32              # images per pipeline block
HALF = 16             # images per conv12/conv3 matmul
NBLK = B_CORE // BLK  # 8 blocks

NW12 = 10             # conv12 windows: in rows 6w..6w+9 -> out rows 6w..6w+5
C3_WINS = [(w, 5 * w, 7, 5) for w in range(5)] + [(5, 25, 5, 3)]
C4_WINS = [(w, 5 * w, 7, 5) for w in range(5)] + [(5, 25, 3, 1)]


# ---------------------------------------------------------------- host prep
def _compose12(c1_w, c2_w):
    """5x5 composite kernel K12[o,i,s,t] of conv2(conv1(x)) (both valid 3x3)."""
    K12 = np.zeros((18, 6, 5, 5), np.float64)
    for a in range(3):
        for ax in range(3):
            for b in range(3):
                for bx in range(3):
                    K12[:, :, a + b, ax + bx] += np.einsum(
                        "oc,ci->oi", c2_w[:, :, a, ax].astype(np.float64),
                        c1_w[:, :, b, bx].astype(np.float64))
    return K12.astype(np.float32)


def _parity_col(r, co):
    """M-column for output row r (0..5), channel co: pair-parity layout."""
    return (r // 2) * 18 + co + (64 if r % 2 else 0)


def _t12(K12, taps):
    """lhsT [len(taps)*60, 118] for the composed conv: row g*60+u*6+ci holds
    tap taps[g] (polyphase: g also selects the shifted input copy); band
    0 <= u-r <= 4; cols pair-parity grouped."""
    T = np.zeros((len(taps) * 60, 118), np.float32)
    for g, t in enumerate(taps):
        for u in range(10):
            for r in range(6):
                s = u - r
                if 0 <= s < 5:
                    T[g * 60 + u * 6:g * 60 + (u + 1) * 6,
                      _parity_col(r, 0):_parity_col(r, 0) + 18] = K12[:, :, s, t].T
    return T


def _toeplitz(w, kx, U, R, cin):
    T = np.zeros((U * cin, R * 18), np.float32)
    for u in range(U):
        for r in range(R):
            ky = u - r
            if 0 <= ky < 3:
                T[u * cin:(u + 1) * cin, r * 18:(r + 1) * 18] = w[:, :, ky, kx].T
    return T


def _pack3(w, U, R, cin):
    return np.concatenate([_toeplitz(w, kx, U, R, cin) for kx in range(3)], axis=1)


def _host_prep(c1_w, c1_b, c2_w, c2_b, c3_w, c3_b, c4_w, c4_b,
               l1_w, l1_b, l2_w, l2_b, ag_w, ag_b):
    p = {}
    K12 = _compose12(c1_w, c2_w)
    # polyphase tile row blocks: [0:60] = plane shifted +1, [60:120] = plain
    p["Te"] = _t12(K12, [2, 0])    # [120, 118]  same-parity plane taps
    p["To"] = _t12(K12, [3, 1])    # [120, 118]  opposite-plane taps
    p["Tc"] = _t12(K12, [4])       # [60, 118]   tap 4 (shifted block @ +1)

    p["T3"] = _pack3(c3_w, 7, 5, 18)          # [126, 270]
    p["T3p"] = _pack3(c3_w, 5, 3, 18)         # [90, 162]
    p["T4"] = _pack3(c4_w, 7, 5, 18) * 0.25   # [126, 270]
    p["T4p"] = _pack3(c4_w, 3, 1, 18) * 0.25  # [54, 54]

    # l1 weights with vertical avgpool folded in
    wl1 = l1_w[:, :3042].reshape(64, 18, 13, 13)
    for w4, _, _, R4 in C4_WINS:
        Wf = np.zeros((R4 * 18, 13, 64), np.float32)
        for r in range(R4):
            y = 5 * w4 + r
            for x2 in range(13):
                Wf[r * 18:(r + 1) * 18, x2, :] = wl1[:, :, y // 2, x2].T
        p[f"Wfc{w4}"] = Wf.reshape(R4 * 18, 13 * 64)
    p["Wst"] = np.ascontiguousarray(l1_w[:, 3042:3044].astype(np.float32).T)

    # fold conv biases into l1 bias (linear chain; constants commute w/ pools)
    c1 = c1_b.astype(np.float64)
    c2 = c2_b + c2_w.sum((2, 3)).astype(np.float64) @ c1
    c3 = c3_b + c3_w.sum((2, 3)).astype(np.float64) @ c2
    c4 = c4_b + c4_w.sum((2, 3)).astype(np.float64) @ c3
    beff = l1_b.astype(np.float64) + l1_w[:, :3042].reshape(64, 18, 169).sum(-1).astype(np.float64) @ c4
    p["beff"] = beff.astype(np.float32).reshape(64, 1)
    p["Wl2"] = np.ascontiguousarray(l2_w.T.astype(np.float32))
    p["bl2"] = l2_b.astype(np.float32).reshape(64, 1)
    p["We"] = np.ascontiguousarray(ag_w.transpose(2, 0, 1).reshape(64, 10)).astype(np.float32)
    p["be0"] = ag_b[0].astype(np.float32).reshape(5, 1)
    p["be1"] = ag_b[1].astype(np.float32).reshape(5, 1)
    p["ones5"] = np.ones((5, 1), np.float32)
    p["rep5"] = np.ones((1, 5), np.float32)
    for k in ["Te", "To", "Tc", "T3", "T3p", "T4", "T4p",
              "Wfc0", "Wfc1", "Wfc2", "Wfc3", "Wfc4", "Wfc5", "Wst"]:
        p[k] = p[k].astype(NP_BF16)
    return p


# ---------------------------------------------------------------- device build
_CACHE = {}

_WSPEC = [
    ("Te", [120, 118], BF16), ("To", [120, 118], BF16),
    ("Tc", [60, 118], BF16),
    ("T3", [126, 270], BF16), ("T3p", [90, 162], BF16),
    ("T4", [126, 270], BF16), ("T4p", [54, 54], BF16),
    ("Wfc0", [90, 832], BF16), ("Wfc1", [90, 832], BF16),
    ("Wfc2", [90, 832], BF16), ("Wfc3", [90, 832], BF16),
    ("Wfc4", [90, 832], BF16), ("Wfc5", [18, 832], BF16),
    ("Wst", [2, 64], BF16),
    ("Wl2", [64, 64], F32R), ("We", [64, 10], F32R),
    ("beff", [64, 1], F32), ("bl2", [64, 1], F32),
    ("be0", [5, 1], F32), ("be1", [5, 1], F32),
    ("ones5", [5, 1], F32R), ("rep5", [1, 5], F32R),
    ("st", [2, B_CORE], BF16), ("sel", [1, B_CORE], F32R),
]


def _build():
    nc = bacc.Bacc("TRN2", debug=False)

    x_ext = nc.declare_dram_parameter("x", [NW12 * 240, B_CORE, 32], BF16,
                                      isOutput=False)
    wparams = {}
    for name, shape, dt in _WSPEC:
        wparams[name] = nc.declare_dram_parameter(name, shape, dt, isOutput=False)
    out_ext = nc.declare_dram_parameter("out", [B_CORE * 5], F32, isOutput=True)

    with tile.TileContext(nc) as tc:
        with (
            tc.tile_pool(name="wp", bufs=1) as wp,
            tc.tile_pool(name="act", bufs=1) as ap_,
            tc.tile_pool(name="ps", bufs=1, space="PSUM") as ps,
        ):
            W = {}
            for name, ext in wparams.items():
                t = wp.tile(list(ext.shape), ext.dtype, name=f"w_{name}")
                nc.sync.dma_start(out=t[...], in_=ext.ap())
                W[name] = t

            fc = {}
            for w4, _, _, R4 in C4_WINS:
                fc[w4] = wp.tile([R4 * 18, 13, B_CORE], BF16, name=f"fc{w4}")

            for blk in range(NBLK):
                ib = slice(blk * BLK, (blk + 1) * BLK)
                # ---- conv12 window loads (polyphase even/odd planes)
                xw = {}
                for w in range(NW12):
                    te = ap_.tile([120, BLK, 32], BF16, name="xwe", tag="xwe", bufs=4)
                    to = ap_.tile([120, BLK, 32], BF16, name="xwo", tag="xwo", bufs=4)
                    nc.sync.dma_start(out=te[...],
                                      in_=x_ext.ap()[w * 240:w * 240 + 120, ib, :])
                    nc.sync.dma_start(out=to[...],
                                      in_=x_ext.ap()[w * 240 + 120:w * 240 + 240, ib, :])
                    xw[w] = (te, to)

                # ---- conv12 matmuls + maxH + maxV
                mv = {}
                for w in range(NW12):
                    mh = ap_.tile([118, BLK, 30], BF16, name="mh", tag="mh", bufs=3)
                    for s in range(2):
                        isl = slice(s * HALF, (s + 1) * HALF)
                        pse = ps.tile([118, HALF, 30], F32, name="ps2e",
                                      tag="psE", bufs=2)
                        pso = ps.tile([118, HALF, 30], F32, name="ps2o",
                                      tag="psO", bufs=2)
                        xe, xo = xw[w]
                        for pt, same, opp, off in ((pse, xe, xo, 0),
                                                   (pso, xo, xe, 1)):
                            nc.tensor.matmul(
                                pt[...], W["Te"][...],
                                same[:, isl, 0:30], start=True, stop=False)
                            nc.tensor.matmul(
                                pt[...], W["To"][...],
                                opp[:, isl, off:off + 30],
                                start=False, stop=False)
                            nc.tensor.matmul(
                                pt[...], W["Tc"][...],
                                same[0:60, isl, 1:31],
                                start=False, stop=True)
                        se = ap_.tile([118, HALF, 30], F32, name="se",
                                      tag="se", bufs=3)
                        nc.scalar.copy(se[...], pse[...])
                        nc.vector.tensor_max(mh[:, isl, :], se[...], pso[...])
                    mho = ap_.tile([54, BLK, 30], BF16, name="mho",
                                   tag="mho", bufs=3)
                    nc.vector.tensor_copy(mho[...], mh[64:118, :, :])
                    mvw = ap_.tile([54, BLK, 30], BF16, name="mv",
                                   tag="mv", bufs=11)
                    nc.vector.tensor_max(mvw[...], mh[0:54, :, :], mho[...])
                    mv[w] = mvw

                # ---- assemble conv3 windows from mv (span gathers), conv3
                c4win = {}
                for w3, prow0, U, R in C3_WINS:
                    c4win[w3] = ap_.tile([126, BLK, 2, 14], BF16, name="c4w",
                                         tag="c4win", bufs=7)
                ge = 0
                for w3, prow0, U, R in C3_WINS:
                    t3 = ap_.tile([126, BLK, 30], BF16, name="c3w",
                                  tag="c3win", bufs=3)
                    u = 0
                    while u < U:
                        k = prow0 + u
                        mw, r = k // 3, k % 3
                        span = 1
                        while u + span < U and (k + span) // 3 == mw:
                            span += 1
                        eng = nc.sync if ge % 2 == 0 else nc.scalar
                        ge += 1
                        eng.dma_start(
                            out=t3[u * 18:(u + span) * 18, :, :],
                            in_=mv[mw][r * 18:(r + span) * 18, :, :])
                        u += span
                    TW = W["T3"] if U == 7 else W["T3p"]
                    for h in range(2):
                        hsl = slice(h * HALF, (h + 1) * HALF)
                        pt = ps.tile([R * 18, HALF, 28], F32, name="ps3",
                                     tag="psA", bufs=2)
                        for kx in range(3):
                            nc.tensor.matmul(
                                pt[...],
                                TW[0:U * 18, kx * R * 18:(kx + 1) * R * 18],
                                t3[0:U * 18, hsl, kx:kx + 28],
                                start=(kx == 0), stop=(kx == 2))
                        nc.scalar.copy(c4win[w3][0:R * 18, hsl, :, :],
                                       pt[...].rearrange("p i (x two) -> p i two x", two=2))
                for w4 in range(5):
                    src = c4win[w4 + 1] if w4 < 4 else c4win[5]
                    nc.gpsimd.dma_start(out=c4win[w4][90:126, :, :, :],
                                        in_=src[0:36, :, :, :])

                # ---- conv4: 6 passes -> psum IS avgH -> copy into FC tiles
                for w4, _, U, R in C4_WINS:
                    TW = W["T4"] if U == 7 else W["T4p"]
                    pt = ps.tile([R * 18, BLK, 13], F32, name="ps4",
                                 tag="psE", bufs=2)
                    n = 0
                    for j in (0, 1):
                        for kx in range(3):
                            plane = (j + kx) % 2
                            off = (j + kx) // 2
                            rhs = c4win[w4][0:U * 18, :, plane, off:off + 13]
                            nc.tensor.matmul(
                                pt[...],
                                TW[0:U * 18, kx * R * 18:(kx + 1) * R * 18],
                                rhs, start=(n == 0), stop=(n == 5))
                            n += 1
                    nc.vector.tensor_copy(fc[w4][:, :, ib],
                                          pt[...].rearrange("p i x -> p x i"))

            # ---------------- FC + routing (all 256 images)
            ph1 = ps.tile([64, B_CORE], F32, name="ph1", tag="psA", bufs=2)
            first = True
            for w4, _, _, R4 in C4_WINS:
                wfc = W[f"Wfc{w4}"][...].rearrange("p (x m) -> p x m", m=64)
                for x2 in range(13):
                    nc.tensor.matmul(ph1[...], wfc[0:R4 * 18, x2, :],
                                     fc[w4][:, x2, :], start=first, stop=False)
                    first = False
            nc.tensor.matmul(ph1[...], W["Wst"][...], W["st"][...],
                             start=False, stop=True)
            h1 = wp.tile([64, B_CORE], F32R, name="h1")
            nc.scalar.activation(h1[...], ph1[...],
                                 mybir.ActivationFunctionType.Tanh,
                                 bias=W["beff"][...])

            ph2 = ps.tile([64, B_CORE], F32, name="ph2", tag="psA", bufs=2)
            nc.tensor.matmul(ph2[...], W["Wl2"][...], h1[...], start=True, stop=True)
            h2 = wp.tile([64, B_CORE], F32R, name="h2")
            nc.scalar.activation(h2[...], ph2[...],
                                 mybir.ActivationFunctionType.Tanh,
                                 bias=W["bl2"][...])

            We_r = W["We"][...].rearrange("p (e m) -> p e m", m=5)
            g = []
            for e in range(2):
                pe = ps.tile([5, B_CORE], F32, name=f"pe{e}", tag="psO", bufs=2)
                nc.tensor.matmul(pe[...], We_r[:, e, :], h2[...],
                                 start=True, stop=True)
                gt = wp.tile([5, B_CORE], F32, name=f"g{e}")
                nc.scalar.activation(gt[...], pe[...],
                                     mybir.ActivationFunctionType.Identity,
                                     bias=W[f"be{e}"][...])
                g.append(gt)

            psel = ps.tile([5, B_CORE], F32, name="psel", tag="psO", bufs=2)
            nc.tensor.matmul(psel[...], W["rep5"][...], W["sel"][...],
                             start=True, stop=True)
            sel5 = wp.tile([5, B_CORE], F32, name="sel5")
            nc.scalar.copy(sel5[...], psel[...])

            dif = wp.tile([5, B_CORE], F32, name="dif")
            nc.vector.tensor_sub(dif[...], g[1][...], g[0][...])
            nc.vector.tensor_mul(dif[...], dif[...], sel5[...])
            lg = wp.tile([5, B_CORE], F32, name="lg")
            nc.vector.tensor_add(lg[...], g[0][...], dif[...])

            E = wp.tile([5, B_CORE], F32R, name="E")
            nc.scalar.activation(E[...], lg[...], mybir.ActivationFunctionType.Exp)
            psum_s = ps.tile([1, B_CORE], F32, name="psum_s", tag="psO", bufs=2)
            nc.tensor.matmul(psum_s[...], W["ones5"][...], E[...],
                             start=True, stop=True)
            s_sb = wp.tile([1, B_CORE], F32, name="s_sb")
            nc.scalar.copy(s_sb[...], psum_s[...])
            r_sb = wp.tile([1, B_CORE], F32R, name="r_sb")
            with nc.allow_low_precision(reason="f32r reciprocal feeding f32r matmul"):
                nc.vector.reciprocal(r_sb[...], s_sb[...])
            pr5 = ps.tile([5, B_CORE], F32, name="pr5", tag="psO", bufs=2)
            nc.tensor.matmul(pr5[...], W["rep5"][...], r_sb[...],
                             start=True, stop=True)
            r5 = wp.tile([5, B_CORE], F32, name="r5")
            nc.scalar.copy(r5[...], pr5[...])
            probs = wp.tile([5, B_CORE], F32, name="probs")
            nc.vector.tensor_mul(probs[...], E[...], r5[...])

            nc.sync.dma_start(
                out=out_ext.ap().rearrange("(b o) -> o b", o=5), in_=probs[...])

    nc.finalize()
    return nc


def _get_nc():
    if "nc" not in _CACHE:
        _CACHE["nc"] = _build()
    return _CACHE["nc"]


# ---------------------------------------------------------------- input prep
def make_x2(states_core):
    """[2400, B_CORE, 32] bf16 polyphase: window w block = [xeP | xoP], each
    [120]: rows 0:60 = plane, rows 60:120 = plane shifted by one element."""
    B = states_core.shape[0]
    out = np.zeros((NW12 * 240, B, 32), dtype=NP_BF16)
    for w in range(NW12):
        sub = states_core[:, :, 6 * w:6 * w + 10, :]           # [B,6,10,64]
        sub = sub.transpose(2, 1, 0, 3)                        # [10,6,B,64]
        for pi, plane in ((0, sub[..., 0::2]), (1, sub[..., 1::2])):
            base = w * 240 + pi * 120
            pl = plane.reshape(60, B, 32).astype(NP_BF16)
            out[base:base + 60, :, 0:31] = pl[:, :, 1:32]   # shifted block
            out[base + 60:base + 120] = pl                   # plain block
    return out


def make_in_maps(p, states, scores, times, agents_np):
    in_maps = []
    for c in range(N_CORES):
        sl = slice(c * B_CORE, (c + 1) * B_CORE)
        m = dict(p)
        m["x"] = make_x2(states[sl])
        m["st"] = np.stack([scores[sl, 0], times[sl, 0]], axis=0).astype(NP_BF16)
        m["sel"] = agents_np[sl].astype(np.float32).reshape(1, B_CORE)
        in_maps.append(m)
    return in_maps


# ---------------------------------------------------------------- entry point
def kernel(states, scores, times, agents,
           c1_w, c1_b, c2_w, c2_b, c3_w, c3_b, c4_w, c4_b,
           l1_w, l1_b, l2_w, l2_b, ag_w, ag_b, _want_trace=False, **_ignore):
    states = np.asarray(states, np.float32)
    scores = np.asarray(scores, np.float32)
    times = np.asarray(times, np.float32)
    agents_np = np.asarray(agents)
    p = _host_prep(np.asarray(c1_w, np.float32), np.asarray(c1_b, np.float32),
                   np.asarray(c2_w, np.float32), np.asarray(c2_b, np.float32),
                   np.asarray(c3_w, np.float32), np.asarray(c3_b, np.float32),
                   np.asarray(c4_w, np.float32), np.asarray(c4_b, np.float32),
                   np.asarray(l1_w, np.float32), np.asarray(l1_b, np.float32),
                   np.asarray(l2_w, np.float32), np.asarray(l2_b, np.float32),
                   np.asarray(ag_w, np.float32), np.asarray(ag_b, np.float32))

    nc = _get_nc()
    in_maps = make_in_maps(p, states, scores, times, agents_np)

    res = run_bass_kernel_spmd(nc, in_maps, list(range(N_CORES)),
                               trace=_want_trace)
    out = np.concatenate([res.results[c]["out"] for c in range(N_CORES)])
    if _want_trace:
        kernel._last = res
    return out


# revision 17
# speedup vs baseline: 1.4512x; 1.4006x over previous
"""Trainium2 Bass kernel for nn_Actor_network (moe_routing).

Data-parallel over 8 NeuronCores: each core processes 256 of the 2048 images
through convs (+pools) -> FC -> 2-expert routing -> softmax.

v3 design:
  - conv1 and conv2 are LINEAR back-to-back (no activation between), so they
    are composed on host into one 5x5 conv (6->18 ch), kernel K12.
  - The composed conv runs as banded-Toeplitz matmuls with kx-PAIR
    replication: K = 2 kx-taps x 10 input rows x 6 ch = 120; the remaining
    kx taps reuse the same SBUF rows at shifted free offsets, giving 3
    accumulating passes (taps {0,1}, {2,3}, {4}).  R=6 output rows per
    window, M = 118 (pair-parity grouped: even-of-pair rows at partition 0,
    odd-of-pair at 64).
  - Each pass is split into even-x / odd-x strided matmuls accumulating in
    two PSUM tiles; horizontal maxpool = ACT copy (psum->SBUF, HW allows
    only one PSUM operand per tensor_tensor) + one DVE tensor_max;
    vertical maxpool = ONE DVE tensor_max with partition-offset operands
    (rows pair-align inside each window since R=6 is even) -> mv tiles.
  - conv3 windows are assembled from mv tiles by span DMA gathers.
  - conv4: 6 passes (3 kx x 2 x-parity) accumulate into ONE psum tile =
    horizontal avgpool for free (0.25 folded into conv4 weights); vertical
    avgpool is folded into the l1 contraction (weights duplicated per row
    pair); pooled features go straight from conv4 psum into persistent
    per-window FC tiles.
  - bf16 activations/weights everywhere in the conv pipeline (PSUM f32).
"""
import numpy as np
import ml_dtypes

import concourse.bass as bass
import concourse.mybir as mybir
from concourse import bacc, tile
from concourse.bass_utils import run_bass_kernel_spmd

F32 = mybir.dt.float32
F32R = mybir.dt.float32r
BF16 = mybir.dt.bfloat16
NP_BF16 = ml_dtypes.bfloat16

N_CORES = 8
B_CORE = 256          # images per core
BLK = 32              # images per pipeline block
HALF = 16             # images per conv12/conv3 matmul
NBLK = B_CORE // BLK  # 8 blocks

NW12 = 10             # conv12 windows: in rows 6w..6w+9 -> out rows 6w..6w+5
C3_WINS = [(w, 5 * w, 7, 5) for w in range(5)] + [(5, 25, 5, 3)]
C4_WINS = [(w, 5 * w, 7, 5) for w in range(5)] + [(5, 25, 3, 1)]


# ---------------------------------------------------------------- host prep
def _compose12(c1_w, c2_w):
    """5x5 composite kernel K12[o,i,s,t] of conv2(conv1(x)) (both valid 3x3)."""
    K12 = np.zeros((18, 6, 5, 5), np.float64)
    for a in range(3):
        for ax in range(3):
            for b in range(3):
                for bx in range(3):
                    K12[:, :, a + b, ax + bx] += np.einsum(
                        "oc,ci->oi", c2_w[:, :, a, ax].astype(np.float64),
                        c1_w[:, :, b, bx].astype(np.float64))
    return K12.astype(np.float32)


def _parity_col(r, co):
    """M-column for output row r (0..5), channel co: pair-parity layout."""
    return (r // 2) * 18 + co + (64 if r % 2 else 0)


def _t12(K12, taps):
    """lhsT [len(taps)*60, 118] for the composed conv: row g*60+u*6+ci holds
    tap taps[g] (polyphase: g also selects the shifted input copy); band
    0 <= u-r <= 4; cols pair-parity grouped."""
    T = np.zeros((len(taps) * 60, 118), np.float32)
    for g, t in enumerate(taps):
        for u in range(10):
            for r in range(6):
                s = u - r
                if 0 <= s < 5:
                    T[g * 60 + u * 6:g * 60 + (u + 1) * 6,
                      _parity_col(r, 0):_parity_col(r, 0) + 18] = K12[:, :, s, t].T
    return T


def _toeplitz(w, kx, U, R, cin):
    T = np.zeros((U * cin, R * 18), np.float32)
    for u in range(U):
        for r in range(R):
            ky = u - r
            if 0 <= ky < 3:
                T[u * cin:(u + 1) * cin, r * 18:(r + 1) * 18] = w[:, :, ky, kx].T
    return T


def _pack3(w, U, R, cin):
    return np.concatenate([_toeplitz(w, kx, U, R, cin) for kx in range(3)], axis=1)


def _host_prep(c1_w, c1_b, c2_w, c2_b, c3_w, c3_b, c4_w, c4_b,
               l1_w, l1_b, l2_w, l2_b, ag_w, ag_b):
    p = {}
    K12 = _compose12(c1_w, c2_w)
    # polyphase tile row blocks: [0:60] = plane shifted +1, [60:120] = plain
    p["Te"] = _t12(K12, [2, 0])    # [120, 118]  same-parity plane taps
    p["To"] = _t12(K12, [3, 1])    # [120, 118]  opposite-plane taps
    p["Tc"] = _t12(K12, [4])       # [60, 118]   tap 4 (shifted block @ +1)

    p["T3"] = _pack3(c3_w, 7, 5, 18)          # [126, 270]
    p["T3p"] = _pack3(c3_w, 5, 3, 18)         # [90, 162]
    p["T4"] = _pack3(c4_w, 7, 5, 18) * 0.25   # [126, 270]
    p["T4p"] = _pack3(c4_w, 3, 1, 18) * 0.25  # [54, 54]

    # l1 weights with vertical avgpool folded in
    wl1 = l1_w[:, :3042].reshape(64, 18, 13, 13)
    for w4, _, _, R4 in C4_WINS:
        Wf = np.zeros((R4 * 18, 13, 64), np.float32)
        for r in range(R4):
            y = 5 * w4 + r
            for x2 in range(13):
                Wf[r * 18:(r + 1) * 18, x2, :] = wl1[:, :, y // 2, x2].T
        p[f"Wfc{w4}"] = Wf.reshape(R4 * 18, 13 * 64)
    p["Wst"] = np.ascontiguousarray(l1_w[:, 3042:3044].astype(np.float32).T)

    # fold conv biases into l1 bias (linear chain; constants commute w/ pools)
    c1 = c1_b.astype(np.float64)
    c2 = c2_b + c2_w.sum((2, 3)).astype(np.float64) @ c1
    c3 = c3_b + c3_w.sum((2, 3)).astype(np.float64) @ c2
    c4 = c4_b + c4_w.sum((2, 3)).astype(np.float64) @ c3
    beff = l1_b.astype(np.float64) + l1_w[:, :3042].reshape(64, 18, 169).sum(-1).astype(np.float64) @ c4
    p["beff"] = beff.astype(np.float32).reshape(64, 1)
    p["Wl2"] = np.ascontiguousarray(l2_w.T.astype(np.float32))
    p["bl2"] = l2_b.astype(np.float32).reshape(64, 1)
    p["We"] = np.ascontiguousarray(ag_w.transpose(2, 0, 1).reshape(64, 10)).astype(np.float32)
    p["be0"] = ag_b[0].astype(np.float32).reshape(5, 1)
    p["be1"] = ag_b[1].astype(np.float32).reshape(5, 1)
    p["ones5"] = np.ones((5, 1), np.float32)
    p["rep5"] = np.ones((1, 5), np.float32)
    for k in ["Te", "To", "Tc", "T3", "T3p", "T4", "T4p",
              "Wfc0", "Wfc1", "Wfc2", "Wfc3", "Wfc4", "Wfc5", "Wst"]:
        p[k] = p[k].astype(NP_BF16)
    return p


# ---------------------------------------------------------------- device build
_CACHE = {}

_WSPEC = [
    ("Te", [120, 118], BF16), ("To", [120, 118], BF16),
    ("Tc", [60, 118], BF16),
    ("T3", [126, 270], BF16), ("T3p", [90, 162], BF16),
    ("T4", [126, 270], BF16), ("T4p", [54, 54], BF16),
    ("Wfc0", [90, 832], BF16), ("Wfc1", [90, 832], BF16),
    ("Wfc2", [90, 832], BF16), ("Wfc3", [90, 832], BF16),
    ("Wfc4", [90, 832], BF16), ("Wfc5", [18, 832], BF16),
    ("Wst", [2, 64], BF16),
    ("Wl2", [64, 64], F32R), ("We", [64, 10], F32R),
    ("beff", [64, 1], F32), ("bl2", [64, 1], F32),
    ("be0", [5, 1], F32), ("be1", [5, 1], F32),
    ("ones5", [5, 1], F32R), ("rep5", [1, 5], F32R),
    ("st", [2, B_CORE], BF16), ("sel", [1, B_CORE], F32R),
]


def _build():
    nc = bacc.Bacc("TRN2", debug=False)

    x_ext = nc.declare_dram_parameter("x", [NW12 * 240, B_CORE, 32], BF16,
                                      isOutput=False)
    wparams = {}
    for name, shape, dt in _WSPEC:
        wparams[name] = nc.declare_dram_parameter(name, shape, dt, isOutput=False)
    out_ext = nc.declare_dram_parameter("out", [B_CORE * 5], F32, isOutput=True)

    with tile.TileContext(nc) as tc:
        with (
            tc.tile_pool(name="wp", bufs=1) as wp,
            tc.tile_pool(name="act", bufs=1) as ap_,
            tc.tile_pool(name="ps", bufs=1, space="PSUM") as ps,
        ):
            W = {}
            for name, ext in wparams.items():
                t = wp.tile(list(ext.shape), ext.dtype, name=f"w_{name}")
                nc.sync.dma_start(out=t[...], in_=ext.ap())
                W[name] = t

            fc = {}
            for w4, _, _, R4 in C4_WINS:
                fc[w4] = wp.tile([R4 * 18, 13, B_CORE], BF16, name=f"fc{w4}")

            for blk in range(NBLK):
                ib = slice(blk * BLK, (blk + 1) * BLK)
                # ---- conv12 window loads (polyphase even/odd planes)
                xw = {}
                for w in range(NW12):
                    te = ap_.tile([120, BLK, 32], BF16, name="xwe", tag="xwe", bufs=4)
                    to = ap_.tile([120, BLK, 32], BF16, name="xwo", tag="xwo", bufs=4)
                    nc.sync.dma_start(out=te[...],
                                      in_=x_ext.ap()[w * 240:w * 240 + 120, ib, :])
                    nc.sync.dma_start(out=to[...],
                                      in_=x_ext.ap()[w * 240 + 120:w * 240 + 240, ib, :])
                    xw[w] = (te, to)

                # ---- conv12 matmuls + maxH + maxV
                mv = {}
                for w in range(NW12):
                    mh = ap_.tile([118, BLK, 30], BF16, name="mh", tag="mh", bufs=3)
                    for s in range(2):
                        isl = slice(s * HALF, (s + 1) * HALF)
                        pse = ps.tile([118, HALF, 30], F32, name="ps2e",
                                      tag="psE", bufs=3)
                        pso = ps.tile([118, HALF, 30], F32, name="ps2o",
                                      tag="psO", bufs=3)
                        xe, xo = xw[w]
                        for pt, same, opp, off in ((pse, xe, xo, 0),
                                                   (pso, xo, xe, 1)):
                            nc.tensor.matmul(
                                pt[...], W["Te"][...],
                                same[:, isl, 0:30], start=True, stop=False)
                            nc.tensor.matmul(
                                pt[...], W["To"][...],
                                opp[:, isl, off:off + 30],
                                start=False, stop=False)
                            nc.tensor.matmul(
                                pt[...], W["Tc"][...],
                                same[0:60, isl, 1:31],
                                start=False, stop=True)
                        se = ap_.tile([118, HALF, 30], F32, name="se",
                                      tag="se", bufs=3)
                        nc.scalar.copy(se[...], pse[...])
                        nc.vector.tensor_max(mh[:, isl, :], se[...], pso[...])
                    mho = ap_.tile([54, BLK, 30], BF16, name="mho",
                                   tag="mho", bufs=3)
                    nc.vector.tensor_copy(mho[...], mh[64:118, :, :])
                    mvw = ap_.tile([54, BLK, 30], BF16, name="mv",
                                   tag="mv", bufs=11)
                    nc.vector.tensor_max(mvw[...], mh[0:54, :, :], mho[...])
                    mv[w] = mvw

                # ---- assemble conv3 windows from mv (span gathers), conv3
                c4win = {}
                for w3, prow0, U, R in C3_WINS:
                    c4win[w3] = ap_.tile([126, BLK, 2, 14], BF16, name="c4w",
                                         tag="c4win", bufs=7)
                ge = 0
                for w3, prow0, U, R in C3_WINS:
                    t3 = ap_.tile([126, BLK, 30], BF16, name="c3w",
                                  tag="c3win", bufs=3)
                    u = 0
                    while u < U:
                        k = prow0 + u
                        mw, r = k // 3, k % 3
                        span = 1
                        while u + span < U and (k + span) // 3 == mw:
                            span += 1
                        eng = nc.sync if ge % 2 == 0 else nc.scalar
                        ge += 1
                        eng.dma_start(
                            out=t3[u * 18:(u + span) * 18, :, :],
                            in_=mv[mw][r * 18:(r + span) * 18, :, :])
                        u += span
                    TW = W["T3"] if U == 7 else W["T3p"]
                    for h in range(2):
                        hsl = slice(h * HALF, (h + 1) * HALF)
                        pt = ps.tile([R * 18, HALF, 28], F32, name="ps3",
                                     tag="psA", bufs=2)
                        for kx in range(3):
                            nc.tensor.matmul(
                                pt[...],
                                TW[0:U * 18, kx * R * 18:(kx + 1) * R * 18],
                                t3[0:U * 18, hsl, kx:kx + 28],
                                start=(kx == 0), stop=(kx == 2))
                        nc.scalar.copy(c4win[w3][0:R * 18, hsl, :, :],
                                       pt[...].rearrange("p i (x two) -> p i two x", two=2))
                for w4 in range(5):
                    src = c4win[w4 + 1] if w4 < 4 else c4win[5]
                    nc.gpsimd.dma_start(out=c4win[w4][90:126, :, :, :],
                                        in_=src[0:36, :, :, :])

                # ---- conv4: 6 passes -> psum IS avgH -> copy into FC tiles
                for w4, _, U, R in C4_WINS:
                    TW = W["T4"] if U == 7 else W["T4p"]
                    pt = ps.tile([R * 18, BLK, 13], F32, name="ps4",
                                 tag="psE", bufs=3)
                    n = 0
                    for j in (0, 1):
                        for kx in range(3):
                            plane = (j + kx) % 2
                            off = (j + kx) // 2
                            rhs = c4win[w4][0:U * 18, :, plane, off:off + 13]
                            nc.tensor.matmul(
                                pt[...],
                                TW[0:U * 18, kx * R * 18:(kx + 1) * R * 18],
                                rhs, start=(n == 0), stop=(n == 5))
                            n += 1
                    nc.vector.tensor_copy(fc[w4][:, :, ib],
                                          pt[...].rearrange("p i x -> p x i"))

            # ---------------- FC + routing (all 256 images)
            ph1 = ps.tile([64, B_CORE], F32, name="ph1", tag="psA", bufs=2)
            first = True
            for w4, _, _, R4 in C4_WINS:
                wfc = W[f"Wfc{w4}"][...].rearrange("p (x m) -> p x m", m=64)
                for x2 in range(13):
                    nc.tensor.matmul(ph1[...], wfc[0:R4 * 18, x2, :],
                                     fc[w4][:, x2, :], start=first, stop=False)
                    first = False
            nc.tensor.matmul(ph1[...], W["Wst"][...], W["st"][...],
                             start=False, stop=True)
            h1 = wp.tile([64, B_CORE], F32R, name="h1")
            nc.scalar.activation(h1[...], ph1[...],
                                 mybir.ActivationFunctionType.Tanh,
                                 bias=W["beff"][...])

            ph2 = ps.tile([64, B_CORE], F32, name="ph2", tag="psA", bufs=2)
            nc.tensor.matmul(ph2[...], W["Wl2"][...], h1[...], start=True, stop=True)
            h2 = wp.tile([64, B_CORE], F32R, name="h2")
            nc.scalar.activation(h2[...], ph2[...],
                                 mybir.ActivationFunctionType.Tanh,
                                 bias=W["bl2"][...])

            We_r = W["We"][...].rearrange("p (e m) -> p e m", m=5)
            g = []
            for e in range(2):
                pe = ps.tile([5, B_CORE], F32, name=f"pe{e}", tag="psO", bufs=3)
                nc.tensor.matmul(pe[...], We_r[:, e, :], h2[...],
                                 start=True, stop=True)
                gt = wp.tile([5, B_CORE], F32, name=f"g{e}")
                nc.scalar.activation(gt[...], pe[...],
                                     mybir.ActivationFunctionType.Identity,
                                     bias=W[f"be{e}"][...])
                g.append(gt)

            psel = ps.tile([5, B_CORE], F32, name="psel", tag="psO", bufs=3)
            nc.tensor.matmul(psel[...], W["rep5"][...], W["sel"][...],
                             start=True, stop=True)
            sel5 = wp.tile([5, B_CORE], F32, name="sel5")
            nc.scalar.copy(sel5[...], psel[...])

            dif = wp.tile([5, B_CORE], F32, name="dif")
            nc.vector.tensor_sub(dif[...], g[1][...], g[0][...])
            nc.vector.tensor_mul(dif[...], dif[...], sel5[...])
            lg = wp.tile([5, B_CORE], F32, name="lg")
            nc.vector.tensor_add(lg[...], g[0][...], dif[...])

            E = wp.tile([5, B_CORE], F32R, name="E")
            nc.scalar.activation(E[...], lg[...], mybir.ActivationFunctionType.Exp)
            psum_s = ps.tile([1, B_CORE], F32, name="psum_s", tag="psO", bufs=3)
            nc.tensor.matmul(psum_s[...], W["ones5"][...], E[...],
                             start=True, stop=True)
            s_sb = wp.tile([1, B_CORE], F32, name="s_sb")
            nc.scalar.copy(s_sb[...], psum_s[...])
            r_sb = wp.tile([1, B_CORE], F32R, name="r_sb")
            with nc.allow_low_precision(reason="f32r reciprocal feeding f32r matmul"):
                nc.vector.reciprocal(r_sb[...], s_sb[...])
            pr5 = ps.tile([5, B_CORE], F32, name="pr5", tag="psO", bufs=3)
            nc.tensor.matmul(pr5[...], W["rep5"][...], r_sb[...],
                             start=True, stop=True)
            r5 = wp.tile([5, B_CORE], F32, name="r5")
            nc.scalar.copy(r5[...], pr5[...])
            probs = wp.tile([5, B_CORE], F32, name="probs")
            nc.vector.tensor_mul(probs[...], E[...], r5[...])

            nc.sync.dma_start(
                out=out_ext.ap().rearrange("(b o) -> o b", o=5), in_=probs[...])

    nc.finalize()
    return nc


def _get_nc():
    if "nc" not in _CACHE:
        _CACHE["nc"] = _build()
    return _CACHE["nc"]


# ---------------------------------------------------------------- input prep
def make_x2(states_core):
    """[2400, B_CORE, 32] bf16 polyphase: window w block = [xeP | xoP], each
    [120]: rows 0:60 = plane, rows 60:120 = plane shifted by one element."""
    B = states_core.shape[0]
    out = np.zeros((NW12 * 240, B, 32), dtype=NP_BF16)
    for w in range(NW12):
        sub = states_core[:, :, 6 * w:6 * w + 10, :]           # [B,6,10,64]
        sub = sub.transpose(2, 1, 0, 3)                        # [10,6,B,64]
        for pi, plane in ((0, sub[..., 0::2]), (1, sub[..., 1::2])):
            base = w * 240 + pi * 120
            pl = plane.reshape(60, B, 32).astype(NP_BF16)
            out[base:base + 60, :, 0:31] = pl[:, :, 1:32]   # shifted block
            out[base + 60:base + 120] = pl                   # plain block
    return out


def make_in_maps(p, states, scores, times, agents_np):
    in_maps = []
    for c in range(N_CORES):
        sl = slice(c * B_CORE, (c + 1) * B_CORE)
        m = dict(p)
        m["x"] = make_x2(states[sl])
        m["st"] = np.stack([scores[sl, 0], times[sl, 0]], axis=0).astype(NP_BF16)
        m["sel"] = agents_np[sl].astype(np.float32).reshape(1, B_CORE)
        in_maps.append(m)
    return in_maps


# ---------------------------------------------------------------- entry point
def kernel(states, scores, times, agents,
           c1_w, c1_b, c2_w, c2_b, c3_w, c3_b, c4_w, c4_b,
           l1_w, l1_b, l2_w, l2_b, ag_w, ag_b, _want_trace=False, **_ignore):
    states = np.asarray(states, np.float32)
    scores = np.asarray(scores, np.float32)
    times = np.asarray(times, np.float32)
    agents_np = np.asarray(agents)
    p = _host_prep(np.asarray(c1_w, np.float32), np.asarray(c1_b, np.float32),
                   np.asarray(c2_w, np.float32), np.asarray(c2_b, np.float32),
                   np.asarray(c3_w, np.float32), np.asarray(c3_b, np.float32),
                   np.asarray(c4_w, np.float32), np.asarray(c4_b, np.float32),
                   np.asarray(l1_w, np.float32), np.asarray(l1_b, np.float32),
                   np.asarray(l2_w, np.float32), np.asarray(l2_b, np.float32),
                   np.asarray(ag_w, np.float32), np.asarray(ag_b, np.float32))

    nc = _get_nc()
    in_maps = make_in_maps(p, states, scores, times, agents_np)

    res = run_bass_kernel_spmd(nc, in_maps, list(range(N_CORES)),
                               trace=_want_trace)
    out = np.concatenate([res.results[c]["out"] for c in range(N_CORES)])
    if _want_trace:
        kernel._last = res
    return out
